# revision 53
# baseline (speedup 1.0000x reference)
"""Trainium2 Bass kernel for nn_Attention_28750511080014 (GQA causal attention).

Reference semantics (replicated exactly, including the noncanonical plain
reshape):
  qkv = x @ w_attn.T                         [B,S,1536]
  q = qkv[..., :1024].reshape(B, 16, S, 64)  # plain reshape, no transpose
  k = qkv[..., 1024:1280].reshape(B, 4, S, 64); v likewise
  causal softmax attention with repeat_interleave(4) on kv heads
  y -> transpose -> [B,S,1024] @ w_proj.T

Key structural fact: the plain reshape makes q-head n's [2048, 64] matrix a
contiguous reinterpretation of rows [n*128,(n+1)*128) of the [2048,1024]
q2d = x @ wq.T matrix; kv-head m similarly uses rows [m*512,(m+1)*512) of
the [2048,256] k2d/v2d matrices. So q-heads {4g..4g+3} and kv-head g only
need x rows [g*512,(g+1)*512).

Sharding: 8 cores = 2 batches x 4 kv-groups. Core (b,g) computes 4 q-heads +
1 kv head and a partial output projection over its 256 y2d columns; host sums
the 4 partials per batch (row-parallel linear unshard).

Layout/engine plan (cost-model-driven):
  - All inputs bf16 (host-converted); matmuls bf16 (full PE rate, no N>=256
    f32r restriction).
  - S_T[k,q] = K-tile.T @ Q.T per (head-pair, k-tile), diagonal tiles
    restricted to the causally-valid q range (wedge skipped in matmul, exp
    and mask); P_T = exp(S_T) on Act (scores O(1), no max subtraction).
  - Boundary causal mask: one [128,2,128] scalar_tensor_tensor on DVE per
    diagonal tile (tril pattern, both heads at once).
  - PV in [q,d] orientation: Y[q,(d|1)] accumulates pt-tile.T @ V1 per
    128-q tile with exact per-tile causal k range (N=65 bf16 matmuls).
  - Normalization: reciprocal of the den column + per-partition
    tensor_scalar multiply (PSUM f32 -> SBUF bf16), then a PE transpose
    (bf16 is_transpose into a bitcast PSUM view) + one 2x-mode DVE copy
    into y2dT[(m,d), q]. V1 likewise from vTs. (DMA XBAR transposes and
    implicit SWDGE read deps proved racy in this stack and are avoided /
    explicitly fenced.)
  - Projection out.T = wpT.T @ y2dT per quarter, metered into PE-light
    units via a deferred-work FIFO; PSUM evac to bf16 on DVE/Act; outputs
    leave via one batched SWDGE DMA per quarter on the idle Pool queue
    with explicit deps on the evac writes.
  - QKV is r-chunked: pair (0,0)'s slice runs pre-attention, the rest
    drains through the FIFO under the Act-paced attention, with explicit
    name deps from the strided evac copies to their S_T/PV consumers (the
    tile dep tracker misses some strided-AP overlaps, which the scheduler
    then reorders across).
"""

import sys
import numpy as np
from contextlib import ExitStack

for _p in ("/opt/trn_rl_repo",):
    if _p not in sys.path:
        sys.path.insert(0, _p)

B, S, H = 2, 2048, 1024
NQ, NKV, HD = 16, 4, 64
GHD = 256          # kv projection width (4 heads * 64)
G = 4              # q heads per kv head == cores per batch
SB = 512           # x rows per core block
W_QKV = H + 2 * GHD  # 1536
NCORES = 8
NH = 4             # local q heads per core
KT = 16            # k-tiles of 128 over S
SCALE = 0.125      # 1/sqrt(64), folded into wq on host

_NC = None


def _build_body(ctx, tc, xT, wT, wpT, mskb, idn, outT):
    import concourse.bass as bass
    import concourse.mybir as mybir

    nc = tc.nc
    dt = mybir.dt
    f32 = dt.float32
    bf16 = dt.bfloat16
    Exp = mybir.ActivationFunctionType.Exp
    mult = mybir.AluOpType.mult

    # ---- pools ----
    cpool = ctx.enter_context(tc.tile_pool(name="consts", bufs=1))
    inp = ctx.enter_context(tc.tile_pool(name="inputs", bufs=1))
    psA = ctx.enter_context(tc.tile_pool(name="psA", bufs=2, space="PSUM"))
    psB = ctx.enter_context(tc.tile_pool(name="psB", bufs=2, space="PSUM"))
    psY = ctx.enter_context(tc.tile_pool(name="psY", bufs=2, space="PSUM"))
    ptp = ctx.enter_context(tc.tile_pool(name="ptp", bufs=8))
    ptq = ctx.enter_context(tc.tile_pool(name="ptq", bufs=24))

    # ---- SBUF input tensors ----
    xT_sb = inp.tile([128, 8, SB], bf16, tag="xt")
    wT_sb = inp.tile([128, 8, W_QKV], bf16, tag="wt")
    wpT_sb = inp.tile([128, 2, H], bf16, tag="wpt")
    mask_sb = cpool.tile([128, 2, 128], bf16, tag="mask")
    idn_sb = cpool.tile([128, 128], bf16, tag="idn")

    nc.sync.dma_start(idn_sb[:, :], idn[:, :])

    # batched input loads: HWDGE charges ~625ns fixed per DMA instruction,
    # so a few big rearranged DMAs beat 36 per-tile ones
    nc.sync.dma_start(xT_sb[:, :, :],
                      xT[0:H, :].rearrange("(ht p) q -> p ht q", p=128))
    nc.sync.dma_start(wT_sb[:, :, 1024:1536],
                      wT[0:H, 1024:1536].rearrange("(ht p) o -> p ht o",
                                                   p=128))
    nc.sync.dma_start(wT_sb[:, :, 0:512],
                      wT[0:H, 0:512].rearrange("(ht p) o -> p ht o", p=128))
    nc.sync.dma_start(wT_sb[:, :, 512:1024],
                      wT[0:H, 512:1024].rearrange("(ht p) o -> p ht o",
                                                  p=128))
    nc.sync.dma_start(mask_sb[:, :, :], mskb[:, :])
    nc.sync.dma_start(wpT_sb[:, :, :],
                      wpT[0:256, :].rearrange("(ct p) o -> p ct o", p=128))

    # ---- qkvT SBUF tensors ----
    # qT3d[64*(j%2)+d, c, 128*j + r'] = q_slab.T[c*64+d, 128*j+r']
    # head j = 2p+m lives at partition half m = j%2; s2 = 16*r_local + c
    qT3d = cpool.tile([128, 16, SB], bf16, tag="qT")
    kTs = cpool.tile([128, S], bf16, tag="kT")   # s2-ordered, dup halves
    vTs = cpool.tile([64, S], bf16, tag="vT")    # s2-ordered
    v1_sb = cpool.tile([128, KT, 128], bf16, tag="v1")  # 128-wide: XBAR dest slices must be 128-aligned
    y2dT = cpool.tile([128, 2, S], bf16, tag="y2dT")

    nc.vector.memset(v1_sb[:, :, HD:HD + 1], 1.0)

    def s2_ap(t3, mlo, r0, nr):
        """[64, nr, 16] AP over t3 partition half mlo, s2-ordered."""
        return t3[64 * mlo:64 * mlo + 64, 0:16, r0:r0 + nr].rearrange(
            "d c r -> d r c")

    # The tile framework does not reliably link XBAR-transpose (and SWDGE
    # read) accesses to their consumers, so those edges are added explicitly
    # by instruction name via add_sync_dependencies_from.
    from bass_rust import InstructionNameOrderedSet as _NameSet
    q_wr = {0: [], 1: []}   # q chunk -> qT3d copy inst names
    v_wr = {}               # rc -> vTs copy inst names
    k_wr = {}               # rc -> kTs copy inst names
    v1_wr = {}              # rc -> v1 copy inst name

    # ---- QKV projection, r-chunked so most of it overlaps attention ----
    # kv chunk rc covers x-rows [128rc,128rc+128) -> k-tiles 4rc..4rc+3;
    # q chunk ch covers x-rows [256ch,256ch+256) -> q-heads 2ch,2ch+1 (pair
    # ch). Pre-attention: kv0 + q chunk 0 (what pair (0,0) needs). The rest
    # drains through the deferred-work FIFO under the Act-paced attention.
    def kv_piece(oc, rc, act_ok):
        def go():
            ps = psB.tile([128, 512], f32, tag="b")
            for ht in range(8):
                nc.tensor.matmul(
                    ps[:, 0:128],
                    wT_sb[:, ht, oc * 128:(oc + 1) * 128],
                    xT_sb[:, ht, 128 * rc:128 * (rc + 1)],
                    start=(ht == 0), stop=(ht == 7),
                )
            s0 = 512 * rc
            for half in range(2):
                src = ps[half * 64:(half + 1) * 64, 0:128]
                if oc < 10:
                    c = 2 * (oc - 8) + half
                    ci = nc.vector.tensor_copy(
                        out=kTs[0:64, s0 + c:s0 + 512:4], in_=src)
                    k_wr.setdefault(rc, []).append(ci.ins.name)
                    if act_ok:
                        ci = nc.scalar.copy(kTs[64:128, s0 + c:s0 + 512:4],
                                            src)
                    else:
                        ci = nc.vector.tensor_copy(
                            out=kTs[64:128, s0 + c:s0 + 512:4], in_=src)
                    k_wr[rc].append(ci.ins.name)
                else:
                    c = 2 * (oc - 10) + half
                    ci = nc.vector.tensor_copy(
                        out=vTs[0:64, s0 + c:s0 + 512:4], in_=src)
                    v_wr.setdefault(rc, []).append(ci.ins.name)
            if oc == 11:
                tpv = psB.tile([128, 512], f32, tag="b")
                tpvb = tpv[:, :].bitcast(bf16)
                for i in range(4):
                    kt = 4 * rc + i
                    ti = nc.tensor.transpose(
                        tpvb[:, 64 * i:64 * (i + 1)],
                        vTs[0:64, 128 * kt:128 * (kt + 1)],
                        idn_sb[0:64, 0:64],
                    )
                    ti.ins.add_sync_dependencies_from(_NameSet(v_wr[rc]))
                ci = nc.vector.tensor_copy(
                    out=v1_sb[:, 4 * rc:4 * rc + 4, 0:HD],
                    in_=tpvb[:, 0:256].rearrange("p (k d) -> p k d", d=64),
                )
                v1_wr[rc] = ci.ins.name
        return go

    def q_piece(oc, ch, act_ok):
        def go():
            ps = psB.tile([128, 512], f32, tag="b")
            for ht in range(8):
                nc.tensor.matmul(
                    ps[:, 0:256],
                    wT_sb[:, ht, oc * 128:(oc + 1) * 128],
                    xT_sb[:, ht, 256 * ch:256 * (ch + 1)],
                    start=(ht == 0), stop=(ht == 7),
                )
            for half in range(2):
                c = 2 * oc + half
                s3 = ps[half * 64:(half + 1) * 64, 0:256].rearrange(
                    "p (b z) -> p b z", z=128)
                for m in range(2):  # block m of this chunk = head j=2ch+m
                    sap = s3[:, m, :]
                    dap = qT3d[64 * m:64 * m + 64, c,
                               256 * ch + 128 * m:256 * ch + 128 * (m + 1)]
                    if act_ok and (half + m) % 2 == 1:
                        ci = nc.scalar.copy(dap, sap)
                    else:
                        ci = nc.vector.tensor_copy(out=dap, in_=sap)
                    q_wr[ch].append(ci.ins.name)
        return go

    for oc in (8, 9, 10, 11):
        kv_piece(oc, 0, True)()
    for oc in range(8):
        q_piece(oc, 0, True)()

    # ---- attention ----
    obq_pool = ctx.enter_context(tc.tile_pool(name="obq", bufs=5))

    def proj_pieces(hq, cols=(0, 512)):
        Q0q = 512 * hq + cols[0]
        ncol = cols[1] - cols[0]
        obq = obq_pool.tile([128, 8, 512], bf16, tag="obq")

        evac_names = []

        def piece(ot):
            def go():
                ctx2 = tc.high_priority(offset=-120)
                ctx2.__enter__()
                pp = psB.tile([128, 512], f32, tag="b")
                for ct in range(2):
                    nc.tensor.matmul(
                        pp[:, 0:ncol],
                        wpT_sb[:, ct, ot * 128:(ot + 1) * 128],
                        y2dT[:, ct, Q0q:Q0q + ncol],
                        start=(ct == 0), stop=(ct == 1),
                    )
                if hq == 3 and ot % 2 == 1:
                    # tail: exp is done, the Act engine is free to share evac
                    ei = nc.scalar.copy(obq[:, ot, 0:ncol], pp[:, 0:ncol])
                else:
                    ei = nc.vector.tensor_copy(out=obq[:, ot, 0:ncol],
                                               in_=pp[:, 0:ncol])
                evac_names.append(ei.ins.name)
                if ot == 7:
                    # One batched SWDGE DMA per quarter(-half) on the
                    # otherwise-idle Pool queue; explicit deps on the evac
                    # writes (SWDGE read deps are not reliably honored).
                    di = nc.gpsimd.dma_start(
                        outT[0:H, Q0q:Q0q + ncol].rearrange(
                            "(ot p) q -> p ot q", p=128),
                        obq[:, :, 0:ncol],
                    )
                    di.ins.add_sync_dependencies_from(_NameSet(list(evac_names)))
                ctx2.__exit__(None, None, None)
            return go
        return [piece(ot) for ot in range(8)]

    def emit_norm_qt(p, h, qt, yqs):
        rr = ptp.tile([128, 2], f32, tag="rr")
        yn = ptp.tile([128, 2, HD], bf16, tag="yn")
        for m in range(2):
            nc.vector.reciprocal(rr[:, m:m + 1],
                                 yqs[m][:, 65 * qt + 64:65 * qt + 65])
            nc.vector.tensor_scalar_mul(
                yn[:, m, :],
                yqs[m][:, 65 * qt:65 * qt + 64],
                rr[:, m:m + 1],
            )
        tp = psB.tile([128, 512], f32, tag="b")
        tpb = tp[:, :].bitcast(bf16)
        nc.tensor.transpose(tpb[:, 0:128], yn[:, :, :], idn_sb[:, :])
        nc.vector.tensor_copy(
            out=y2dT[:, p, 512 * h + 128 * qt:512 * h + 128 * (qt + 1)],
            in_=tpb[:, 0:128],
        )

    def emit_norm(p, h):
        yqs = norm_pend.pop((p, h), None)
        if yqs is None:
            return
        rr = ptp.tile([128, 8], f32, tag="rr8")
        for m in range(2):
            nc.vector.reciprocal(rr[:, 4 * m:4 * m + 4],
                                 yqs[m][:, 64:260:65])
        yn = ptp.tile([128, 4, 2, HD], bf16, tag="yn4")
        tp = psB.tile([128, 512], f32, tag="b")
        tpb = tp[:, :].bitcast(bf16)
        for qt in range(4):
            for m in range(2):
                nc.vector.tensor_scalar_mul(
                    yn[:, qt, m, :],
                    yqs[m][:, 65 * qt:65 * qt + 64],
                    rr[:, 4 * m + qt:4 * m + qt + 1],
                )
            nc.tensor.transpose(tpb[:, 128 * qt:128 * (qt + 1)],
                                yn[:, qt, :, :], idn_sb[:, :])
        nc.vector.tensor_copy(
            out=y2dT[:, p, 512 * h:512 * (h + 1)],
            in_=tpb[:, 0:512],
        )

    def make_burst(yqs, pts, h, qt):
        # PV for q-tile qt of this pair: contiguous accumulation group over
        # all causally-needed k-tiles (exactly one open group per PSUM bank).
        kt_stop = 4 * h + qt

        def go():
            for m in range(2):
                for kt in range(kt_stop + 1):
                    mi = nc.tensor.matmul(
                        yqs[m][:, 65 * qt:65 * qt + 65],
                        pts[kt][:, 512 * m + 128 * qt:512 * m + 128 * (qt + 1)],
                        v1_sb[:, kt, 0:65],
                        start=(kt == 0), stop=(kt == kt_stop),
                    )
                    mi.ins.add_sync_dependencies_from(
                        _NameSet([v1_wr[kt // 4]]))
        return go

    norm_pend = {}
    # Two deferred-work queues, drained per kt unit: `urgent` (PV bursts +
    # norms — they free PSUM banks and pt buffers, so they always go first)
    # and `filler` (QKV remainder chunks + projection pieces, metered into
    # PE-light units). Preload filler with the rest of QKV, ordered by when
    # the pair sequence below first needs it.
    urgent = []
    filler = []
    filler.extend(kv_piece(oc, 1, False) for oc in (8, 9, 10, 11))
    filler.extend(q_piece(oc, 1, False) for oc in range(8))
    for rc in (2, 3):
        filler.extend(kv_piece(oc, rc, False) for oc in (8, 9, 10, 11))

    for p, h in ((0, 0), (0, 1), (0, 2), (1, 0), (0, 3),
                 (1, 1), (1, 2), (1, 3)):
            kt_max = 4 * h + 3
            # force-drain filler until everything this pair reads has been
            # emitted (q chunk p, kv chunks 0..h) — both for legal emission
            # order and so the explicit dep sets below are complete
            while (len(q_wr[p]) < 32
                   or any(rc not in v1_wr for rc in range(h + 1))):
                filler.pop(0)()
            yqs = None
            pts = []

            for kt in range(kt_max + 1):
                mm = kt - 4 * h      # >= 0 on the block diagonal
                q0 = 128 * max(0, mm)
                st = psA.tile([128, 1024], f32, tag="st")
                for m in range(2):
                    j = 2 * p + m
                    mi = nc.tensor.matmul(
                        st[:, 512 * m + q0:512 * m + 512],
                        kTs[64 * m:64 * m + 64, 128 * kt:128 * (kt + 1)],
                        s2_ap(qT3d, m, 128 * j + 32 * h + q0 // 16,
                              32 - q0 // 16),
                        start=True, stop=True,
                    )
                    # strided-AP write->read edges the dep tracker misses
                    mi.ins.add_sync_dependencies_from(
                        _NameSet(q_wr[p] + k_wr[kt // 4]))
                pt = ptq.tile([128, 1024], bf16, tag="pt")
                pts.append(pt)
                st3 = st[:, :].rearrange("p (m q) -> p m q", q=512)
                pt3 = pt[:, :].rearrange("p (m q) -> p m q", q=512)
                nc.scalar.activation(pt3[:, :, q0:512], st3[:, :, q0:512],
                                     Exp)
                if mm >= 0:  # diagonal: tril-mask the boundary 128-block
                    nc.vector.scalar_tensor_tensor(
                        out=pt3[:, :, q0:q0 + 128],
                        in0=pt3[:, :, q0:q0 + 128],
                        scalar=1.0,
                        in1=mask_sb[:, :, :],
                        op0=mult, op1=mult,
                    )
                # urgent first (frees PSUM + pt rings), then metered filler.
                # The previous pair's last burst + norm drain here, AFTER
                # this pair's first S_T/exp (so Act never waits on them) but
                # BEFORE the new yq tiles are allocated (the pool must see
                # every access to the recycled bank already emitted).
                while urgent:
                    urgent.pop(0)()
                if yqs is None:
                    yqA = psY.tile([128, 512], f32, tag="yq")  # head 2p
                    yqB = psY.tile([128, 512], f32, tag="yq")  # head 2p+1
                    yqs = (yqA, yqB)
                    norm_pend[(p, h)] = yqs
                nfill = 2 if len(filler) > 8 else 1
                for _ in range(min(nfill, len(filler))):
                    filler.pop(0)()
                if mm >= 0:
                    urgent.append(make_burst(yqs, pts, h, mm))
                    if p == 1 and h == 3:
                        # tail: normalize each q-tile as soon as its PV
                        # group closes; fire each proj half once its half
                        # of y2dT is complete
                        urgent.append(
                            _norm_closure(emit_norm_qt, (1, 3, mm, yqs)))
                        if mm == 1:
                            filler.extend(proj_pieces(3, (0, 256)))
                        elif mm == 3:
                            filler.extend(proj_pieces(3, (256, 512)))

            # queue this pair's norm right behind its last PV burst; queue
            # quarter h's projection once both pairs' norms are in line
            if not (p == 1 and h == 3):
                urgent.append(_norm_closure(emit_norm, (p, h)))
            if p == 1 and h < 3:
                filler.extend(proj_pieces(h))
    # drain remaining bursts/norms/pieces
    norm_pend.pop((1, 3), None)
    for go in urgent:
        go()
    for go in filler:
        go()
    return qT3d, kTs, v1_sb, y2dT


def _norm_closure(emit_norm, key):
    def go():
        emit_norm(*key)
    return go


def _emit_pv(nc, v1_sb, h, pend):
    yqs, kt, pt = pend
    mm = kt - 4 * h
    for m in range(2):
        for qt in range(max(0, mm), 4):
            nc.tensor.matmul(
                yqs[m][:, 65 * qt:65 * qt + 65],
                pt[:, 512 * m + 128 * qt:512 * m + 128 * (qt + 1)],
                v1_sb[:, kt, 0:65],
                start=(kt == 0), stop=(kt == 4 * h + qt),
            )


def _build():
    import concourse.tile as tile
    from concourse import bacc
    import concourse.mybir as mybir

    dt = mybir.dt
    nc = bacc.Bacc("TRN2", target_bir_lowering=False, debug=False,
                   num_devices=NCORES)
    xT = nc.dram_tensor("xt", [H, SB], dt.bfloat16, kind="ExternalInput").ap()
    wT = nc.dram_tensor("wt", [H, W_QKV], dt.bfloat16,
                        kind="ExternalInput").ap()
    wpT = nc.dram_tensor("wpt", [GHD, H], dt.bfloat16,
                         kind="ExternalInput").ap()
    mskb = nc.dram_tensor("mskb", [128, 256], dt.bfloat16,
                          kind="ExternalInput").ap()
    idn = nc.dram_tensor("idn", [128, 128], dt.bfloat16,
                         kind="ExternalInput").ap()
    outT = nc.dram_tensor("outt", [H, S], dt.bfloat16,
                          kind="ExternalOutput").ap()

    with tile.TileContext(nc) as tc, ExitStack() as ctx:
        ctx.enter_context(
            nc.allow_low_precision(reason="bf16 rounding is intentional"))
        _build_body(ctx, tc, xT, wT, wpT, mskb, idn, outT)
    nc.compile()
    return nc


def _get_nc():
    global _NC
    if _NC is None:
        _NC = _build()
    return _NC


def _host_inputs(x, w_attn, w_proj):
    import ml_dtypes
    bf = ml_dtypes.bfloat16
    x = np.asarray(x, np.float32)
    w_attn = np.asarray(w_attn, np.float32)
    w_proj = np.asarray(w_proj, np.float32)
    wq = w_attn[:H] * SCALE
    wT_np = np.ascontiguousarray(
        np.concatenate([wq, w_attn[H:]], axis=0).T).astype(bf)  # [1024, 1536]

    idn_np = np.eye(128, dtype=np.float32).astype(bf)
    tri = (np.arange(128)[:, None] <= np.arange(128)[None, :])
    mskb = np.ascontiguousarray(
        np.broadcast_to(tri[:, None, :], (128, 2, 128)).reshape(128, 256)
    ).astype(bf)

    in_maps = []
    for c in range(NCORES):
        b, g = c // 4, c % 4
        xT = np.ascontiguousarray(x[b, g * SB:(g + 1) * SB, :].T).astype(bf)
        wpT = np.ascontiguousarray(
            w_proj[:, g * GHD:(g + 1) * GHD].T).astype(bf)
        in_maps.append({"xt": xT, "wt": wT_np, "wpt": wpT, "mskb": mskb,
                        "idn": idn_np})
    return in_maps


def _gather(results):
    out = np.zeros((B, S, H), np.float32)
    for c in range(NCORES):
        b = c // 4
        out[b] += results[c]["outt"].T.astype(np.float32)
    return out


def kernel(x, w_attn, w_proj):
    from concourse.bass_utils import run_bass_kernel_spmd
    nc = _get_nc()
    in_maps = _host_inputs(x, w_attn, w_proj)
    res = run_bass_kernel_spmd(nc, in_maps, core_ids=list(range(NCORES)))
    return _gather(res.results)


# revision 54
# speedup vs baseline: 1.0019x; 1.0019x over previous
"""Trainium2 Bass kernel for nn_Attention_28750511080014 (GQA causal attention).

Reference semantics (replicated exactly, including the noncanonical plain
reshape):
  qkv = x @ w_attn.T                         [B,S,1536]
  q = qkv[..., :1024].reshape(B, 16, S, 64)  # plain reshape, no transpose
  k = qkv[..., 1024:1280].reshape(B, 4, S, 64); v likewise
  causal softmax attention with repeat_interleave(4) on kv heads
  y -> transpose -> [B,S,1024] @ w_proj.T

Key structural fact: the plain reshape makes q-head n's [2048, 64] matrix a
contiguous reinterpretation of rows [n*128,(n+1)*128) of the [2048,1024]
q2d = x @ wq.T matrix; kv-head m similarly uses rows [m*512,(m+1)*512) of
the [2048,256] k2d/v2d matrices. So q-heads {4g..4g+3} and kv-head g only
need x rows [g*512,(g+1)*512).

Sharding: 8 cores = 2 batches x 4 kv-groups. Core (b,g) computes 4 q-heads +
1 kv head and a partial output projection over its 256 y2d columns; host sums
the 4 partials per batch (row-parallel linear unshard).

Layout/engine plan (cost-model-driven):
  - All inputs bf16 (host-converted); matmuls bf16 (full PE rate, no N>=256
    f32r restriction).
  - S_T[k,q] = K-tile.T @ Q.T per (head-pair, k-tile), diagonal tiles
    restricted to the causally-valid q range (wedge skipped in matmul, exp
    and mask); P_T = exp(S_T) on Act (scores O(1), no max subtraction).
  - Boundary causal mask: one [128,2,128] scalar_tensor_tensor on DVE per
    diagonal tile (tril pattern, both heads at once).
  - PV in [q,d] orientation: Y[q,(d|1)] accumulates pt-tile.T @ V1 per
    128-q tile with exact per-tile causal k range (N=65 bf16 matmuls).
  - Normalization: reciprocal of the den column + per-partition
    tensor_scalar multiply (PSUM f32 -> SBUF bf16), then a PE transpose
    (bf16 is_transpose into a bitcast PSUM view) + one 2x-mode DVE copy
    into y2dT[(m,d), q]. V1 likewise from vTs. (DMA XBAR transposes and
    implicit SWDGE read deps proved racy in this stack and are avoided /
    explicitly fenced.)
  - Projection out.T = wpT.T @ y2dT per quarter, metered into PE-light
    units via a deferred-work FIFO; PSUM evac to bf16 on DVE/Act; outputs
    leave via one batched SWDGE DMA per quarter on the idle Pool queue
    with explicit deps on the evac writes.
  - QKV is r-chunked: pair (0,0)'s slice runs pre-attention, the rest
    drains through the FIFO under the Act-paced attention, with explicit
    name deps from the strided evac copies to their S_T/PV consumers (the
    tile dep tracker misses some strided-AP overlaps, which the scheduler
    then reorders across).
"""

import sys
import numpy as np
from contextlib import ExitStack

for _p in ("/opt/trn_rl_repo",):
    if _p not in sys.path:
        sys.path.insert(0, _p)

B, S, H = 2, 2048, 1024
NQ, NKV, HD = 16, 4, 64
GHD = 256          # kv projection width (4 heads * 64)
G = 4              # q heads per kv head == cores per batch
SB = 512           # x rows per core block
W_QKV = H + 2 * GHD  # 1536
NCORES = 8
NH = 4             # local q heads per core
KT = 16            # k-tiles of 128 over S
SCALE = 0.125      # 1/sqrt(64), folded into wq on host

_NC = None


def _build_body(ctx, tc, xT, wT, wpT, mskb, idn, outT):
    import concourse.bass as bass
    import concourse.mybir as mybir

    nc = tc.nc
    dt = mybir.dt
    f32 = dt.float32
    bf16 = dt.bfloat16
    Exp = mybir.ActivationFunctionType.Exp
    mult = mybir.AluOpType.mult

    # ---- pools ----
    cpool = ctx.enter_context(tc.tile_pool(name="consts", bufs=1))
    inp = ctx.enter_context(tc.tile_pool(name="inputs", bufs=1))
    psA = ctx.enter_context(tc.tile_pool(name="psA", bufs=2, space="PSUM"))
    psB = ctx.enter_context(tc.tile_pool(name="psB", bufs=2, space="PSUM"))
    psY = ctx.enter_context(tc.tile_pool(name="psY", bufs=2, space="PSUM"))
    ptp = ctx.enter_context(tc.tile_pool(name="ptp", bufs=8))
    ptq = ctx.enter_context(tc.tile_pool(name="ptq", bufs=24))

    # ---- SBUF input tensors ----
    xT_sb = inp.tile([128, 8, SB], bf16, tag="xt")
    wT_sb = inp.tile([128, 8, W_QKV], bf16, tag="wt")
    wpT_sb = inp.tile([128, 2, H], bf16, tag="wpt")
    mask_sb = cpool.tile([128, 2, 128], bf16, tag="mask")
    idn_sb = cpool.tile([128, 128], bf16, tag="idn")

    nc.sync.dma_start(idn_sb[:, :], idn[:, :])

    # batched input loads: HWDGE charges ~625ns fixed per DMA instruction,
    # so a few big rearranged DMAs beat 36 per-tile ones
    nc.sync.dma_start(xT_sb[:, :, :],
                      xT[0:H, :].rearrange("(ht p) q -> p ht q", p=128))
    nc.sync.dma_start(wT_sb[:, :, 1024:1536],
                      wT[0:H, 1024:1536].rearrange("(ht p) o -> p ht o",
                                                   p=128))
    nc.sync.dma_start(wT_sb[:, :, 0:512],
                      wT[0:H, 0:512].rearrange("(ht p) o -> p ht o", p=128))
    nc.sync.dma_start(wT_sb[:, :, 512:1024],
                      wT[0:H, 512:1024].rearrange("(ht p) o -> p ht o",
                                                  p=128))
    nc.sync.dma_start(mask_sb[:, :, :], mskb[:, :])
    nc.sync.dma_start(wpT_sb[:, :, :],
                      wpT[0:256, :].rearrange("(ct p) o -> p ct o", p=128))

    # ---- qkvT SBUF tensors ----
    # qT3d[64*(j%2)+d, c, 128*j + r'] = q_slab.T[c*64+d, 128*j+r']
    # head j = 2p+m lives at partition half m = j%2; s2 = 16*r_local + c
    qT3d = cpool.tile([128, 16, SB], bf16, tag="qT")
    kTs = cpool.tile([128, S], bf16, tag="kT")   # s2-ordered, dup halves
    vTs = cpool.tile([64, S], bf16, tag="vT")    # s2-ordered
    v1_sb = cpool.tile([128, KT, 128], bf16, tag="v1")  # 128-wide: XBAR dest slices must be 128-aligned
    y2dT = cpool.tile([128, 2, S], bf16, tag="y2dT")

    nc.vector.memset(v1_sb[:, :, HD:HD + 1], 1.0)

    def s2_ap(t3, mlo, r0, nr):
        """[64, nr, 16] AP over t3 partition half mlo, s2-ordered."""
        return t3[64 * mlo:64 * mlo + 64, 0:16, r0:r0 + nr].rearrange(
            "d c r -> d r c")

    # The tile framework does not reliably link XBAR-transpose (and SWDGE
    # read) accesses to their consumers, so those edges are added explicitly
    # by instruction name via add_sync_dependencies_from.
    from bass_rust import InstructionNameOrderedSet as _NameSet
    q_wr = {0: [], 1: []}   # q chunk -> qT3d copy inst names
    v_wr = {}               # rc -> vTs copy inst names
    k_wr = {}               # rc -> kTs copy inst names
    v1_wr = {}              # rc -> v1 copy inst name

    # ---- QKV projection, r-chunked so most of it overlaps attention ----
    # kv chunk rc covers x-rows [128rc,128rc+128) -> k-tiles 4rc..4rc+3;
    # q chunk ch covers x-rows [256ch,256ch+256) -> q-heads 2ch,2ch+1 (pair
    # ch). Pre-attention: kv0 + q chunk 0 (what pair (0,0) needs). The rest
    # drains through the deferred-work FIFO under the Act-paced attention.
    def kv_piece(oc, rc, act_ok):
        def go():
            ps = psB.tile([128, 512], f32, tag="b")
            for ht in range(8):
                nc.tensor.matmul(
                    ps[:, 0:128],
                    wT_sb[:, ht, oc * 128:(oc + 1) * 128],
                    xT_sb[:, ht, 128 * rc:128 * (rc + 1)],
                    start=(ht == 0), stop=(ht == 7),
                )
            s0 = 512 * rc
            for half in range(2):
                src = ps[half * 64:(half + 1) * 64, 0:128]
                if oc < 10:
                    c = 2 * (oc - 8) + half
                    ci = nc.vector.tensor_copy(
                        out=kTs[0:64, s0 + c:s0 + 512:4], in_=src)
                    k_wr.setdefault(rc, []).append(ci.ins.name)
                    if act_ok:
                        ci = nc.scalar.copy(kTs[64:128, s0 + c:s0 + 512:4],
                                            src)
                    else:
                        ci = nc.vector.tensor_copy(
                            out=kTs[64:128, s0 + c:s0 + 512:4], in_=src)
                    k_wr[rc].append(ci.ins.name)
                else:
                    c = 2 * (oc - 10) + half
                    ci = nc.vector.tensor_copy(
                        out=vTs[0:64, s0 + c:s0 + 512:4], in_=src)
                    v_wr.setdefault(rc, []).append(ci.ins.name)
            if oc == 11:
                tpv = psB.tile([128, 512], f32, tag="b")
                tpvb = tpv[:, :].bitcast(bf16)
                for i in range(4):
                    kt = 4 * rc + i
                    ti = nc.tensor.transpose(
                        tpvb[:, 64 * i:64 * (i + 1)],
                        vTs[0:64, 128 * kt:128 * (kt + 1)],
                        idn_sb[0:64, 0:64],
                    )
                    ti.ins.add_sync_dependencies_from(_NameSet(v_wr[rc]))
                ci = nc.vector.tensor_copy(
                    out=v1_sb[:, 4 * rc:4 * rc + 4, 0:HD],
                    in_=tpvb[:, 0:256].rearrange("p (k d) -> p k d", d=64),
                )
                v1_wr[rc] = ci.ins.name
        return go

    def q_piece(oc, ch, act_ok):
        def go():
            ps = psB.tile([128, 512], f32, tag="b")
            for ht in range(8):
                nc.tensor.matmul(
                    ps[:, 0:256],
                    wT_sb[:, ht, oc * 128:(oc + 1) * 128],
                    xT_sb[:, ht, 256 * ch:256 * (ch + 1)],
                    start=(ht == 0), stop=(ht == 7),
                )
            for half in range(2):
                c = 2 * oc + half
                s3 = ps[half * 64:(half + 1) * 64, 0:256].rearrange(
                    "p (b z) -> p b z", z=128)
                for m in range(2):  # block m of this chunk = head j=2ch+m
                    sap = s3[:, m, :]
                    dap = qT3d[64 * m:64 * m + 64, c,
                               256 * ch + 128 * m:256 * ch + 128 * (m + 1)]
                    if act_ok and (half + m) % 2 == 1:
                        ci = nc.scalar.copy(dap, sap)
                    else:
                        ci = nc.vector.tensor_copy(out=dap, in_=sap)
                    q_wr[ch].append(ci.ins.name)
        return go

    for oc in (8, 9, 10, 11):
        kv_piece(oc, 0, True)()
    for oc in range(8):
        q_piece(oc, 0, True)()

    # ---- attention ----
    obq_pool = ctx.enter_context(tc.tile_pool(name="obq", bufs=5))

    def proj_pieces(hq, cols=(0, 512)):
        Q0q = 512 * hq + cols[0]
        ncol = cols[1] - cols[0]
        obq = obq_pool.tile([128, 8, 512], bf16, tag="obq")

        evac_names = []

        def piece(ot):
            def go():
                ctx2 = tc.high_priority(offset=-40)
                ctx2.__enter__()
                pp = psB.tile([128, 512], f32, tag="b")
                for ct in range(2):
                    nc.tensor.matmul(
                        pp[:, 0:ncol],
                        wpT_sb[:, ct, ot * 128:(ot + 1) * 128],
                        y2dT[:, ct, Q0q:Q0q + ncol],
                        start=(ct == 0), stop=(ct == 1),
                    )
                if hq == 3 and ot % 2 == 1:
                    # tail: exp is done, the Act engine is free to share evac
                    ei = nc.scalar.copy(obq[:, ot, 0:ncol], pp[:, 0:ncol])
                else:
                    ei = nc.vector.tensor_copy(out=obq[:, ot, 0:ncol],
                                               in_=pp[:, 0:ncol])
                evac_names.append(ei.ins.name)
                if ot == 7:
                    # One batched SWDGE DMA per quarter(-half) on the
                    # otherwise-idle Pool queue; explicit deps on the evac
                    # writes (SWDGE read deps are not reliably honored).
                    di = nc.gpsimd.dma_start(
                        outT[0:H, Q0q:Q0q + ncol].rearrange(
                            "(ot p) q -> p ot q", p=128),
                        obq[:, :, 0:ncol],
                    )
                    di.ins.add_sync_dependencies_from(_NameSet(list(evac_names)))
                ctx2.__exit__(None, None, None)
            return go
        return [piece(ot) for ot in range(8)]

    def emit_norm_qt(p, h, qt, yqs):
        rr = ptp.tile([128, 2], f32, tag="rr")
        yn = ptp.tile([128, 2, HD], bf16, tag="yn")
        for m in range(2):
            nc.vector.reciprocal(rr[:, m:m + 1],
                                 yqs[m][:, 65 * qt + 64:65 * qt + 65])
            nc.vector.tensor_scalar_mul(
                yn[:, m, :],
                yqs[m][:, 65 * qt:65 * qt + 64],
                rr[:, m:m + 1],
            )
        tp = psB.tile([128, 512], f32, tag="b")
        tpb = tp[:, :].bitcast(bf16)
        nc.tensor.transpose(tpb[:, 0:128], yn[:, :, :], idn_sb[:, :])
        nc.vector.tensor_copy(
            out=y2dT[:, p, 512 * h + 128 * qt:512 * h + 128 * (qt + 1)],
            in_=tpb[:, 0:128],
        )

    def emit_norm(p, h):
        yqs = norm_pend.pop((p, h), None)
        if yqs is None:
            return
        rr = ptp.tile([128, 8], f32, tag="rr8")
        for m in range(2):
            nc.vector.reciprocal(rr[:, 4 * m:4 * m + 4],
                                 yqs[m][:, 64:260:65])
        yn = ptp.tile([128, 4, 2, HD], bf16, tag="yn4")
        tp = psB.tile([128, 512], f32, tag="b")
        tpb = tp[:, :].bitcast(bf16)
        for qt in range(4):
            for m in range(2):
                nc.vector.tensor_scalar_mul(
                    yn[:, qt, m, :],
                    yqs[m][:, 65 * qt:65 * qt + 64],
                    rr[:, 4 * m + qt:4 * m + qt + 1],
                )
            nc.tensor.transpose(tpb[:, 128 * qt:128 * (qt + 1)],
                                yn[:, qt, :, :], idn_sb[:, :])
        nc.vector.tensor_copy(
            out=y2dT[:, p, 512 * h:512 * (h + 1)],
            in_=tpb[:, 0:512],
        )

    def make_burst(yqs, pts, h, qt):
        # PV for q-tile qt of this pair: contiguous accumulation group over
        # all causally-needed k-tiles (exactly one open group per PSUM bank).
        kt_stop = 4 * h + qt

        def go():
            for m in range(2):
                for kt in range(kt_stop + 1):
                    mi = nc.tensor.matmul(
                        yqs[m][:, 65 * qt:65 * qt + 65],
                        pts[kt][:, 512 * m + 128 * qt:512 * m + 128 * (qt + 1)],
                        v1_sb[:, kt, 0:65],
                        start=(kt == 0), stop=(kt == kt_stop),
                    )
                    mi.ins.add_sync_dependencies_from(
                        _NameSet([v1_wr[kt // 4]]))
        return go

    norm_pend = {}
    # Two deferred-work queues, drained per kt unit: `urgent` (PV bursts +
    # norms — they free PSUM banks and pt buffers, so they always go first)
    # and `filler` (QKV remainder chunks + projection pieces, metered into
    # PE-light units). Preload filler with the rest of QKV, ordered by when
    # the pair sequence below first needs it.
    urgent = []
    filler = []
    filler.extend(kv_piece(oc, 1, False) for oc in (8, 9, 10, 11))
    filler.extend(q_piece(oc, 1, False) for oc in range(8))
    for rc in (2, 3):
        filler.extend(kv_piece(oc, rc, False) for oc in (8, 9, 10, 11))

    for p, h in ((0, 0), (0, 1), (0, 2), (1, 0), (0, 3),
                 (1, 1), (1, 2), (1, 3)):
            kt_max = 4 * h + 3
            # force-drain filler until everything this pair reads has been
            # emitted (q chunk p, kv chunks 0..h) — both for legal emission
            # order and so the explicit dep sets below are complete
            while (len(q_wr[p]) < 32
                   or any(rc not in v1_wr for rc in range(h + 1))):
                filler.pop(0)()
            yqs = None
            pts = []

            for kt in range(kt_max + 1):
                mm = kt - 4 * h      # >= 0 on the block diagonal
                q0 = 128 * max(0, mm)
                st = psA.tile([128, 1024], f32, tag="st")
                for m in range(2):
                    j = 2 * p + m
                    mi = nc.tensor.matmul(
                        st[:, 512 * m + q0:512 * m + 512],
                        kTs[64 * m:64 * m + 64, 128 * kt:128 * (kt + 1)],
                        s2_ap(qT3d, m, 128 * j + 32 * h + q0 // 16,
                              32 - q0 // 16),
                        start=True, stop=True,
                    )
                    # strided-AP write->read edges the dep tracker misses
                    mi.ins.add_sync_dependencies_from(
                        _NameSet(q_wr[p] + k_wr[kt // 4]))
                pt = ptq.tile([128, 1024], bf16, tag="pt")
                pts.append(pt)
                st3 = st[:, :].rearrange("p (m q) -> p m q", q=512)
                pt3 = pt[:, :].rearrange("p (m q) -> p m q", q=512)
                nc.scalar.activation(pt3[:, :, q0:512], st3[:, :, q0:512],
                                     Exp)
                if mm >= 0:  # diagonal: tril-mask the boundary 128-block
                    nc.vector.scalar_tensor_tensor(
                        out=pt3[:, :, q0:q0 + 128],
                        in0=pt3[:, :, q0:q0 + 128],
                        scalar=1.0,
                        in1=mask_sb[:, :, :],
                        op0=mult, op1=mult,
                    )
                # urgent first (frees PSUM + pt rings), then metered filler.
                # The previous pair's last burst + norm drain here, AFTER
                # this pair's first S_T/exp (so Act never waits on them) but
                # BEFORE the new yq tiles are allocated (the pool must see
                # every access to the recycled bank already emitted).
                while urgent:
                    urgent.pop(0)()
                if yqs is None:
                    yqA = psY.tile([128, 512], f32, tag="yq")  # head 2p
                    yqB = psY.tile([128, 512], f32, tag="yq")  # head 2p+1
                    yqs = (yqA, yqB)
                    norm_pend[(p, h)] = yqs
                nfill = 2 if len(filler) > 8 else 1
                for _ in range(min(nfill, len(filler))):
                    filler.pop(0)()
                if mm >= 0:
                    urgent.append(make_burst(yqs, pts, h, mm))
                    if p == 1 and h == 3:
                        # tail: normalize each q-tile as soon as its PV
                        # group closes; fire each proj half once its half
                        # of y2dT is complete
                        urgent.append(
                            _norm_closure(emit_norm_qt, (1, 3, mm, yqs)))
                        if mm == 1:
                            filler.extend(proj_pieces(3, (0, 256)))
                        elif mm == 3:
                            filler.extend(proj_pieces(3, (256, 512)))

            # queue this pair's norm right behind its last PV burst; queue
            # quarter h's projection once both pairs' norms are in line
            if not (p == 1 and h == 3):
                urgent.append(_norm_closure(emit_norm, (p, h)))
            if p == 1 and h < 3:
                filler.extend(proj_pieces(h))
    # drain remaining bursts/norms/pieces
    norm_pend.pop((1, 3), None)
    for go in urgent:
        go()
    for go in filler:
        go()
    return qT3d, kTs, v1_sb, y2dT


def _norm_closure(emit_norm, key):
    def go():
        emit_norm(*key)
    return go


def _emit_pv(nc, v1_sb, h, pend):
    yqs, kt, pt = pend
    mm = kt - 4 * h
    for m in range(2):
        for qt in range(max(0, mm), 4):
            nc.tensor.matmul(
                yqs[m][:, 65 * qt:65 * qt + 65],
                pt[:, 512 * m + 128 * qt:512 * m + 128 * (qt + 1)],
                v1_sb[:, kt, 0:65],
                start=(kt == 0), stop=(kt == 4 * h + qt),
            )


def _build():
    import concourse.tile as tile
    from concourse import bacc
    import concourse.mybir as mybir

    dt = mybir.dt
    nc = bacc.Bacc("TRN2", target_bir_lowering=False, debug=False,
                   num_devices=NCORES)
    xT = nc.dram_tensor("xt", [H, SB], dt.bfloat16, kind="ExternalInput").ap()
    wT = nc.dram_tensor("wt", [H, W_QKV], dt.bfloat16,
                        kind="ExternalInput").ap()
    wpT = nc.dram_tensor("wpt", [GHD, H], dt.bfloat16,
                         kind="ExternalInput").ap()
    mskb = nc.dram_tensor("mskb", [128, 256], dt.bfloat16,
                          kind="ExternalInput").ap()
    idn = nc.dram_tensor("idn", [128, 128], dt.bfloat16,
                         kind="ExternalInput").ap()
    outT = nc.dram_tensor("outt", [H, S], dt.bfloat16,
                          kind="ExternalOutput").ap()

    with tile.TileContext(nc) as tc, ExitStack() as ctx:
        ctx.enter_context(
            nc.allow_low_precision(reason="bf16 rounding is intentional"))
        _build_body(ctx, tc, xT, wT, wpT, mskb, idn, outT)
    nc.compile()
    return nc


def _get_nc():
    global _NC
    if _NC is None:
        _NC = _build()
    return _NC


def _host_inputs(x, w_attn, w_proj):
    import ml_dtypes
    bf = ml_dtypes.bfloat16
    x = np.asarray(x, np.float32)
    w_attn = np.asarray(w_attn, np.float32)
    w_proj = np.asarray(w_proj, np.float32)
    wq = w_attn[:H] * SCALE
    wT_np = np.ascontiguousarray(
        np.concatenate([wq, w_attn[H:]], axis=0).T).astype(bf)  # [1024, 1536]

    idn_np = np.eye(128, dtype=np.float32).astype(bf)
    tri = (np.arange(128)[:, None] <= np.arange(128)[None, :])
    mskb = np.ascontiguousarray(
        np.broadcast_to(tri[:, None, :], (128, 2, 128)).reshape(128, 256)
    ).astype(bf)

    in_maps = []
    for c in range(NCORES):
        b, g = c // 4, c % 4
        xT = np.ascontiguousarray(x[b, g * SB:(g + 1) * SB, :].T).astype(bf)
        wpT = np.ascontiguousarray(
            w_proj[:, g * GHD:(g + 1) * GHD].T).astype(bf)
        in_maps.append({"xt": xT, "wt": wT_np, "wpt": wpT, "mskb": mskb,
                        "idn": idn_np})
    return in_maps


def _gather(results):
    out = np.zeros((B, S, H), np.float32)
    for c in range(NCORES):
        b = c // 4
        out[b] += results[c]["outt"].T.astype(np.float32)
    return out


def kernel(x, w_attn, w_proj):
    from concourse.bass_utils import run_bass_kernel_spmd
    nc = _get_nc()
    in_maps = _host_inputs(x, w_attn, w_proj)
    res = run_bass_kernel_spmd(nc, in_maps, core_ids=list(range(NCORES)))
    return _gather(res.results)
